# revision 7
# baseline (speedup 1.0000x reference)
"""CrossAttentionGate kernel for Trainium2, 8 NeuronCores — v2.

Problem: B=4 batches of single-head spatial cross-attention:
    q = Wq@gate + bq          [B,64,N]   (N = 64*64 = 4096)
    k = Wk@skip + bk          [B,64,N]
    v = Wv@skip + bv          [B,256,N]
    attn = softmax_j(q^T k)   [B,N,N]
    out = gamma * (v @ attn^T) + skip

Sharding: 8 cores = 4 batches x 2 query-halves. Each core computes its
batch's k/v in full (duplicated across the 2 cores of a batch - cheap)
and attends for its 2048 query positions.

Math simplifications (exact up to rounding):
  - bk drops out of softmax (per-row constant shift).
  - No row-max subtraction: logits are O(+-50) here, exp() is fine in fp32.
  - bv commutes past softmax (rows sum to 1): folded into the residual as
    gamma*bv on host.
  - gamma is folded into Wv on host, so the epilogue scale is just 1/sums.
  - skip is COLUMN-PERMUTED on host so each core's own query-half columns
    come first: k/v are order-invariant in j, and the residual slice
    becomes a static slice of the resident skip tile (no separate skipr
    DMA).

Engine layout per 512-query stripe (32 key j-tiles):
  PE : paired logit matmuls (adjacent row-group pairs on partitions
       0-63/64-127 overlap in the array), 2 accumulating out matmuls per
       j-tile (bf16, full speed), final sums reduction + 1/sums broadcast.
  ACT: exp (PSUM->SBUF, bf16 out). The out matmuls are emitted `pipe`
       S-pairs behind the logit matmuls so the PE never blocks ACT's
       input data; st PSUM tiles are 6-buffered (p_sums/p_rb share the
       same bank pool) to allow that lookahead within 8 PSUM banks.
  DVE: softmax-sum accumulation (bf16, 2x mode) + epilogue.
  POOL: softmax-sum accumulation of every pool_every-th j-tile.
All four stripes run under one hardware loop so stripe epilogues overlap
the next stripe's logit matmuls.
"""

import contextlib

import numpy as np

import concourse.bass as bass
import concourse.tile as tile
from concourse import bacc, mybir
from concourse.bass_utils import run_bass_kernel_spmd

F32 = mybir.dt.float32
F32R = mybir.dt.float32r
BF16 = mybir.dt.bfloat16
F16 = mybir.dt.float16
AF = mybir.ActivationFunctionType
ALU = mybir.AluOpType

B, CG, CS, INTER, H, W = 4, 512, 256, 64, 64, 64
KSC = CS // 128
N = H * W            # 4096 spatial positions
NCORES = 8
NI = N // 2          # 2048 query positions per core
NJ = N               # full key/value length per core

# fp16 I/O per core: gate 2.10MB + skip 2.10MB + weights 0.23MB in,
# out 1.05MB back
DMA_BYTES_PER_CORE = 5.474e6


def _build_program(hw_loop_proj=0, hw_loop_inner=0, pipe=2, st_bufs=6,
                   p_bufs=10, pair_st=True, vt_on_act=8, exp_warm=True,
                   parts='full', pool_every=4, eb2=False,
                   k_on_act=True, q_on_act=False):
    nc = bacc.Bacc(
        "TRN2", target_bir_lowering=False, debug=False, num_devices=NCORES
    )
    d_gate = nc.dram_tensor("gate", [CG, NI], F16, kind="ExternalInput").ap()
    d_skip = nc.dram_tensor("skip", [CS, NJ], F16, kind="ExternalInput").ap()
    d_wqt = nc.dram_tensor("wqt", [CG, INTER], F16, kind="ExternalInput").ap()
    d_wkt = nc.dram_tensor("wkt", [CS, INTER], F16, kind="ExternalInput").ap()
    d_wvt = nc.dram_tensor("wvt", [CS, CS], F16, kind="ExternalInput").ap()
    d_bq = nc.dram_tensor("bq", [INTER, 1], F32, kind="ExternalInput").ap()
    d_gbv = nc.dram_tensor("gbv", [128, KSC], F32, kind="ExternalInput").ap()
    d_ones_c = nc.dram_tensor("ones_c", [128, 1], F32, kind="ExternalInput").ap()
    d_ones_r = nc.dram_tensor("ones_r", [1, 128], F32, kind="ExternalInput").ap()
    d_out = nc.dram_tensor("out", [CS, NI], F16, kind="ExternalOutput").ap()

    KG = CG // 128   # 4 gate channel tiles
    KS = CS // 128   # 2 skip channel tiles
    JT = NJ // 128   # 32 key tiles
    NT = NI // 512   # 4 query column stripes

    q_parts = 128 if pair_st else INTER

    with tile.TileContext(nc) as tc:
        with (
            tc.tile_pool(name="res", bufs=1) as res,      # long-lived tensors
            tc.tile_pool(name="stream", bufs=4) as stream,  # P tiles
            tc.tile_pool(name="epi", bufs=2) as epi,
        ):
            # ---- load everything (activations/weights fp16) ----
            wqt_t = []
            for kk in range(KG):
                t = res.tile([128, INTER], F16, tag=f"wqt{kk}", name=f"wqt{kk}")
                nc.sync.dma_start(t[:], d_wqt[kk * 128:(kk + 1) * 128, :])
                wqt_t.append(t)
            wkt_t = []
            for ss in range(KS):
                t = res.tile([128, INTER], F16, tag=f"wkt{ss}", name=f"wkt{ss}")
                nc.sync.dma_start(t[:], d_wkt[ss * 128:(ss + 1) * 128, :])
                wkt_t.append(t)
            wvt_t = []
            for ss in range(KS):
                t = res.tile([128, CS], F16, tag=f"wvt{ss}", name=f"wvt{ss}")
                nc.sync.dma_start(t[:], d_wvt[ss * 128:(ss + 1) * 128, :])
                wvt_t.append(t)
            bq_t = res.tile([INTER, 1], F32, tag="bq")
            nc.sync.dma_start(bq_t[:], d_bq[:])
            gbv_t = res.tile([128, KSC], F32, tag="gbv")
            nc.sync.dma_start(gbv_t[:], d_gbv[:])
            ones_c = res.tile([128, 1], F32R, tag="ones_c")
            nc.sync.dma_start(ones_c[:], d_ones_c[:].bitcast(F32R))
            ones_cb = res.tile([128, 1], BF16, tag="ones_cb")
            nc.vector.tensor_copy(ones_cb[:], ones_c[:])
            ones_r = res.tile([1, 128], F32R, tag="ones_r")
            nc.sync.dma_start(ones_r[:], d_ones_r[:].bitcast(F32R))
            # big activations: skip (feeds k and vT and residual) before gate
            skip_t = []
            for ss in range(KS):
                t = res.tile([128, NJ], F16, tag=f"skip{ss}", name=f"skip{ss}")
                nc.sync.dma_start(t[:], d_skip[ss * 128:(ss + 1) * 128, :])
                skip_t.append(t)
            gate_t = []
            for kk in range(KG):
                t = res.tile([128, NI], F16, tag=f"gate{kk}", name=f"gate{kk}")
                nc.sync.dma_start(t[:], d_gate[kk * 128:(kk + 1) * 128, :])
                gate_t.append(t)

            q_sb = res.tile([q_parts, NI], F32R, tag="q_sb")
            k_sb = res.tile([q_parts, NJ], F32R, tag="k_sb")
            vt_sb = [
                res.tile([128, CS], BF16, tag=f"vt{jt}", name=f"vt{jt}")
                for jt in range(JT)
            ]
            if exp_warm:
                # one tiny exp before the loops so the ACT table load for
                # Exp isn't re-issued inside the hardware loop body
                wtile = epi.tile([1, 1], F32, tag="wtile")
                nc.scalar.activation(wtile[:], bq_t[0:1, 0:1], AF.Exp)

            # ---- projections ----
            proj_ctx = (tc.For_i(0, hw_loop_proj, 1)
                        if hw_loop_proj else contextlib.nullcontext())
            with proj_ctx:
               with tc.tile_pool(name="ps_proj", bufs=2, space="PSUM") as ps_proj:
                   # q[d,i] = sum_g WqT[g,d] gate[g,i] + bq
                   for n in range(NT):
                       pq = ps_proj.tile([INTER, 512], F32, tag="pq")
                       for kk in range(KG):
                           nc.tensor.matmul(
                               pq[:],
                               wqt_t[kk][:],
                               gate_t[kk][:, n * 512:(n + 1) * 512],
                               start=(kk == 0),
                               stop=(kk == KG - 1),
                           )
                       if q_on_act:
                           nc.scalar.activation(
                               q_sb[0:INTER, n * 512:(n + 1) * 512], pq[:],
                               AF.Identity, bias=bq_t[:, 0:1],
                           )
                       else:
                           nc.vector.tensor_scalar(
                               q_sb[0:INTER, n * 512:(n + 1) * 512], pq[:],
                               bq_t[:, 0:1], None, op0=ALU.add,
                           )

                   # k[d,j] = sum_s WkT[s,d] skip[s,j]   (bk drops out)
                   for n in range(NJ // 512):
                       pk = ps_proj.tile([INTER, 512], F32, tag="pk")
                       for ss in range(KS):
                           nc.tensor.matmul(
                               pk[:],
                               wkt_t[ss][:],
                               skip_t[ss][:, n * 512:(n + 1) * 512],
                               start=(ss == 0),
                               stop=(ss == KS - 1),
                           )
                       if k_on_act:
                           nc.scalar.copy(
                               k_sb[0:INTER, n * 512:(n + 1) * 512], pk[:]
                           )
                       else:
                           nc.vector.tensor_copy(
                               k_sb[0:INTER, n * 512:(n + 1) * 512], pk[:]
                           )

                   if pair_st:
                       # duplicate q/k into partitions 64..127 for row-group
                       # paired logit matmuls (DMA: engines cannot write
                       # partition-shifted)
                       nc.sync.dma_start(q_sb[INTER:2 * INTER, :],
                                         q_sb[0:INTER, :])
                       nc.sync.dma_start(k_sb[INTER:2 * INTER, :],
                                         k_sb[0:INTER, :])
                   # vT[j,c] = sum_s skip[s,j] WvT[s,c]  (wvt pre-scaled by
                   # gamma on host)
                   for jt in range(JT):
                       pv = ps_proj.tile([128, CS], F32, tag="pv")
                       for ss in range(KS):
                           nc.tensor.matmul(
                               pv[:],
                               skip_t[ss][:, jt * 128:(jt + 1) * 128],
                               wvt_t[ss][:],
                               start=(ss == 0),
                               stop=(ss == KS - 1),
                           )
                       if jt % 8 < (vt_on_act * 8) // JT:
                           nc.scalar.copy(vt_sb[jt][:], pv[:])
                       else:
                           nc.vector.tensor_copy(vt_sb[jt][:], pv[:])

            # ---- attention, one 512-wide query stripe at a time ----
            with tc.tile_pool(name="ps_attn", bufs=1, space="PSUM") as ps:
                inner_ctx = (tc.For_i(0, hw_loop_inner, 1)
                             if hw_loop_inner else contextlib.nullcontext())
                with inner_ctx:
                  for n in range(NT):
                    if True:
                        qsl = q_sb[:, n * 512:(n + 1) * 512]
                        p_out = [
                            ps.tile([128, 512], F32, tag=f"out{ct}",
                                    name=f"p_out{ct}")
                            for ct in range(KS)
                        ]
                        p_sums_t = ps.tile(
                            [128, 1024] if eb2 else [128, 512], F32,
                            tag="st2" if eb2 else "st",
                            bufs=st_bufs, name="p_sums_t")
                        p_sums = p_sums_t[0:1, 0:512]
                        acc_d = epi.tile([128, 512], BF16, tag="acc_d")
                        acc_p = epi.tile([128, 512], F32R, tag="acc_p")

                        def consume(jt, P):
                            if parts == 'noout':
                                return
                            first = jt == 0
                            last = jt == JT - 1
                            for ct in range(KS):
                                nc.tensor.matmul(
                                    p_out[ct][:],
                                    vt_sb[jt][:, ct * 128:(ct + 1) * 128],
                                    P[:],
                                    start=first,
                                    stop=last,
                                )
                            if parts == 'nosums':
                                return
                            # softmax sums: POOL takes every pool_every-th
                            # tile, DVE (bf16, 2x mode) the rest
                            if pool_every and jt % pool_every == pool_every - 1:
                                if jt == pool_every - 1:
                                    nc.gpsimd.tensor_copy(acc_p[:], P[:])
                                else:
                                    nc.gpsimd.tensor_tensor(
                                        acc_p[:], acc_p[:], P[:], op=ALU.add
                                    )
                            else:
                                if first:
                                    nc.vector.tensor_copy(acc_d[:], P[:])
                                else:
                                    with nc.allow_low_precision(
                                        reason="bf16 softmax-sum partial"
                                    ):
                                        nc.vector.tensor_tensor(
                                            acc_d[:], acc_d[:], P[:],
                                            op=ALU.add,
                                        )

                        # emit S matmuls in adjacent pairs so the two PE
                        # row-groups (partitions 0-63 / 64-127) overlap
                        pending = []
                        for jp in range(JT // 2):
                          if eb2:
                            # one [128,1024] st tile (2 PSUM banks) per
                            # S-pair, a single wide exp over both halves
                            p_st2 = ps.tile([128, 1024], F32, tag="st2",
                                            bufs=st_bufs, name="st2")
                            for u in range(2):
                                jt = 2 * jp + u
                                lo = (jt % 2) * INTER if pair_st else 0
                                nc.tensor.matmul(
                                    p_st2[:, u * 512:(u + 1) * 512],
                                    k_sb[lo:lo + INTER,
                                         jt * 128:(jt + 1) * 128],
                                    qsl[lo:lo + INTER, :],
                                    start=True,
                                    stop=True,
                                )
                            P2 = stream.tile([128, 1024], BF16, tag="P2",
                                             bufs=p_bufs)
                            nc.scalar.activation(P2[:], p_st2[:], AF.Exp)
                            for u in range(2):
                                pending.append((2 * jp + u,
                                                P2[:, u * 512:(u + 1) * 512]))
                            while len(pending) > 2 * pipe:
                                consume(*pending.pop(0))
                          else:
                            Ps = []
                            for u in range(2):
                                jt = 2 * jp + u
                                lo = (jt % 2) * INTER if pair_st else 0
                                p_st = ps.tile([128, 512], F32, tag="st",
                                               bufs=st_bufs, name=f"st{u}")
                                nc.tensor.matmul(
                                    p_st[:],
                                    k_sb[lo:lo + INTER,
                                         jt * 128:(jt + 1) * 128],
                                    qsl[lo:lo + INTER, :],
                                    start=True,
                                    stop=True,
                                )
                                Ps.append((jt, p_st))
                            for jt, p_st in Ps:
                                P = stream.tile([128, 512], BF16, tag="P",
                                                bufs=p_bufs)
                                nc.scalar.activation(P[:], p_st[:], AF.Exp)
                                pending.append((jt, P))
                            while len(pending) > 2 * pipe:
                                consume(*pending.pop(0))
                        for item in pending:
                            consume(*item)

                        if parts == 'noout':
                            continue
                        # sums = ones^T (acc_d + acc_p), via two accumulating
                        # PE matmuls
                        if parts == 'nosums':
                            nc.tensor.matmul(p_sums, ones_c[:],
                                             skip_t[0][:, 0:512],
                                             start=True, stop=True)
                        elif pool_every:
                            nc.tensor.matmul(p_sums, ones_cb[:], acc_d[:],
                                             start=True, stop=False)
                            nc.tensor.matmul(p_sums, ones_c[:], acc_p[:],
                                             start=False, stop=True)
                        else:
                            nc.tensor.matmul(p_sums, ones_cb[:], acc_d[:],
                                             start=True, stop=True)
                        # epilogue: out = (1/sums) * p_out + (skip + gamma*bv)
                        rec = epi.tile([1, 512], F32R, tag="rec")
                        with nc.allow_low_precision(reason="f32r 1/sums"):
                            nc.vector.reciprocal(rec[:], p_sums)
                        p_rb_t = ps.tile(
                            [128, 1024] if eb2 else [128, 512], F32,
                            tag="st2" if eb2 else "st",
                            bufs=st_bufs, name="p_rb")
                        p_rb = p_rb_t[:, 0:512]
                        nc.tensor.matmul(p_rb, ones_r[:], rec[:],
                                         start=True, stop=True)
                        rb_sb = epi.tile([128, 512], F32, tag="rb_sb")
                        nc.vector.tensor_copy(rb_sb[:], p_rb)
                        for ct in range(KS):
                            t0 = epi.tile([128, 512], F32, tag="t0")
                            nc.vector.tensor_tensor(
                                t0[:], p_out[ct][:], rb_sb[:], op=ALU.mult
                            )
                            out_t = epi.tile([128, 512], F16, tag="out_t")
                            with nc.allow_low_precision(
                                reason="fp16 output store"
                            ):
                                nc.vector.scalar_tensor_tensor(
                                    out_t[:],
                                    t0[:],
                                    gbv_t[:, ct:ct + 1],
                                    skip_t[ct][:, n * 512:(n + 1) * 512],
                                    op0=ALU.add,
                                    op1=ALU.add,
                                )
                            nc.sync.dma_start(
                                d_out[ct * 128:(ct + 1) * 128,
                                      n * 512:(n + 1) * 512],
                                out_t[:],
                            )
    nc.compile()
    return nc


_PROGRAM_CACHE = None
BUILD_KWARGS = {}


def kernel(gate, skip, Wq, bq, Wk, bk, Wv, bv, gamma):
    global _PROGRAM_CACHE
    gate = np.ascontiguousarray(np.asarray(gate, dtype=np.float32)).reshape(B, CG, N)
    skip = np.ascontiguousarray(np.asarray(skip, dtype=np.float32)).reshape(B, CS, N)
    Wq = np.asarray(Wq, dtype=np.float32)
    bq = np.asarray(bq, dtype=np.float32)
    Wk = np.asarray(Wk, dtype=np.float32)
    Wv = np.asarray(Wv, dtype=np.float32)
    bv = np.asarray(bv, dtype=np.float32)
    gamma = np.asarray(gamma, dtype=np.float32)

    if _PROGRAM_CACHE is None:
        _PROGRAM_CACHE = _build_program(**BUILD_KWARGS)
    nc = _PROGRAM_CACHE

    wqt = np.ascontiguousarray(Wq.T)                  # [CG, INTER]
    wkt = np.ascontiguousarray(Wk.T)                  # [CS, INTER]
    wvt = np.ascontiguousarray(Wv.T * gamma[0])       # [CS, CS], gamma folded
    bq_c = np.ascontiguousarray(bq.reshape(INTER, 1))
    gbv = np.ascontiguousarray((gamma[0] * bv).reshape(KSC, 128).T)
    ones_c = np.ones((128, 1), np.float32)
    ones_r = np.ones((1, 128), np.float32)

    in_maps = []
    for core in range(NCORES):
        b, h = divmod(core, 2)
        isl = slice(h * NI, (h + 1) * NI)
        osl = slice((1 - h) * NI, (2 - h) * NI)
        # own query-half columns first so the residual is a static slice;
        # j-order is irrelevant to k/v/attention
        skip_perm = np.concatenate([skip[b, :, isl], skip[b, :, osl]], axis=1)
        in_maps.append(
            {
                "gate": np.ascontiguousarray(gate[b, :, isl]).astype(np.float16),
                "skip": np.ascontiguousarray(skip_perm).astype(np.float16),
                "wqt": wqt.astype(np.float16),
                "wkt": wkt.astype(np.float16),
                "wvt": wvt.astype(np.float16),
                "bq": bq_c,
                "gbv": gbv,
                "ones_c": ones_c,
                "ones_r": ones_r,
            }
        )

    res = run_bass_kernel_spmd(nc, in_maps, list(range(NCORES)))

    out = np.empty((B, CS, N), np.float32)
    for core in range(NCORES):
        b, h = divmod(core, 2)
        out[b, :, h * NI:(h + 1) * NI] = res.results[core]["out"].astype(np.float32)
    return out.reshape(B, CS, H, W)
